# revision 1
# baseline (speedup 1.0000x reference)
"""Depthwise conv1d (128 channels, 128 taps, SAME) + softplus on 8 TRN2 cores.

Strategy: data-parallel over the batch dim (16 -> 2 per core). Per channel the
conv is expressed as two banded-Toeplitz matmuls on the tensor engine with the
weight matrices stationary: for output block b (128 timesteps),
    y[b*128+i] = sum_p W1[p,i] * x[b*128-64+p] + sum_p W2[p,i] * x[b*128+64+p]
with W1[p,i] = w[p-i-1], W2[p,i] = w[p+127-i] (zero outside [0,128)).
The Toeplitz tables for all channels are built on the host from `kernels` and
embedded in the NEFF. Activations stream through SBUF as fp16 (rhs), PSUM
accumulates fp32, and softplus(y) = Ln(Exp(y) + 1) runs on the scalar engine
(softplus has no table in this build; conv outputs are well within exp range).
"""
import numpy as np
import concourse.mybir as mybir
from concourse import bacc
from concourse.tile import TileContext
from concourse.bass_utils import run_bass_kernel_spmd

AF = mybir.ActivationFunctionType
N_CORES = 8
B, T, C, K = 16, 32768, 128, 128
B_LOCAL = B // N_CORES


def _build_ws(kernels_np: np.ndarray, dtype=np.float16):
    w = kernels_np[:, 0, :].astype(np.float32)  # [k, c]
    p = np.arange(128)[:, None, None]
    i = np.arange(128)[None, :, None]
    c = np.arange(128)[None, None, :]
    k1 = p - i - 1
    k2 = p + 127 - i
    cb = np.broadcast_to(c, (128, 128, 128))
    W1 = np.where((k1 >= 0) & (k1 < K), w[np.clip(k1, 0, K - 1), cb], 0.0)
    W2 = np.where((k2 >= 0) & (k2 < K), w[np.clip(k2, 0, K - 1), cb], 0.0)
    return (W1.reshape(128, -1).astype(dtype), W2.reshape(128, -1).astype(dtype))


def build_nc(ws1_np, ws2_np, b_local=B_LOCAL, Tn=T, NB=64, GRP=32,
             num_devices=N_CORES):
    nblk = Tn // 128
    assert nblk % NB == 0
    npass_b = nblk // NB
    f16, f32 = mybir.dt.float16, mybir.dt.float32

    nc = bacc.Bacc("TRN2", target_bir_lowering=False, debug=False,
                   num_devices=num_devices)
    x = nc.dram_tensor("x", [b_local, Tn, C], f32, kind="ExternalInput")
    y = nc.dram_tensor("y", [b_local, Tn, C], f32, kind="ExternalOutput")
    ws1_d = nc.inline_tensor(ws1_np, "ws1")
    ws2_d = nc.inline_tensor(ws2_np, "ws2")

    with TileContext(nc) as tc:
        with (
            tc.tile_pool(name="wpool", bufs=1) as wpool,
            tc.tile_pool(name="xpool", bufs=2) as xpool,
            tc.tile_pool(name="ypool", bufs=2) as ypool,
            tc.tile_pool(name="epool", bufs=2) as epool,
            tc.tile_pool(name="ppool", bufs=2, space="PSUM") as ppool,
        ):
            ws1 = wpool.tile([128, 128 * 128], f16, tag="ws1")
            ws2 = wpool.tile([128, 128 * 128], f16, tag="ws2")
            nc.sync.dma_start(out=ws1[:, :], in_=ws1_d.ap())
            nc.sync.dma_start(out=ws2[:, :], in_=ws2_d.ap())
            ws1_3 = ws1.rearrange("p (i c) -> p i c", c=128)
            ws2_3 = ws2.rearrange("p (i c) -> p i c", c=128)

            for bb in range(b_local):
                for P in range(npass_b):
                    B0 = P * NB
                    t_lo = B0 * 128 - 64
                    xt = xpool.tile([128, (NB + 1) * 128], f16, tag="x")
                    first, last = P == 0, P == npass_b - 1
                    if first:
                        nc.vector.memset(xt[0:64, 0:128], 0.0)
                        nc.gpsimd.dma_start(out=xt[64:128, 0:128], in_=x[bb, 0:64, :])
                    else:
                        nc.gpsimd.dma_start(
                            out=xt[:, 0:128], in_=x[bb, t_lo:t_lo + 128, :])
                    nc.gpsimd.dma_start(
                        out=xt.rearrange("p (j c) -> p j c", c=128)[:, 1:NB, :],
                        in_=x[bb, t_lo + 128:t_lo + NB * 128, :].rearrange(
                            "(j p) c -> p j c", p=128),
                    )
                    if last:
                        nc.gpsimd.dma_start(
                            out=xt[0:64, NB * 128:(NB + 1) * 128],
                            in_=x[bb, Tn - 64:Tn, :])
                        nc.vector.memset(xt[64:128, NB * 128:(NB + 1) * 128], 0.0)
                    else:
                        nc.gpsimd.dma_start(
                            out=xt[:, NB * 128:(NB + 1) * 128],
                            in_=x[bb, t_lo + NB * 128:t_lo + (NB + 1) * 128, :])
                    x3 = xt.rearrange("p (j c) -> p j c", c=128)
                    yt = ypool.tile([128, NB * 128], f32, tag="y")
                    y3 = yt.rearrange("p (j c) -> p c j", c=128)  # [p, c, j]
                    for G in range(C // GRP):
                        ps = ppool.tile([128, GRP * NB], f32, tag="ps")
                        et = epool.tile([128, GRP * NB], f32, tag="e")
                        for u in range(GRP):
                            ch = G * GRP + u
                            nc.tensor.matmul(ps[:, u * NB:(u + 1) * NB],
                                             ws1_3[:, :, ch], x3[:, 0:NB, ch],
                                             start=True, stop=False)
                            nc.tensor.matmul(ps[:, u * NB:(u + 1) * NB],
                                             ws2_3[:, :, ch], x3[:, 1:NB + 1, ch],
                                             start=False, stop=True)
                        nc.scalar.activation(et[:, :], ps[:, :], AF.Exp)
                        nc.scalar.activation(
                            y3[:, G * GRP:(G + 1) * GRP, :],
                            et.rearrange("p (u j) -> p u j", j=NB),
                            AF.Ln, bias=1.0)
                    nc.sync.dma_start(
                        out=y[bb, B0 * 128:(B0 + NB) * 128, :].rearrange(
                            "(j p) c -> p j c", p=128),
                        in_=yt.rearrange("p (j c) -> p j c", c=128)[:, :, :])
    nc.finalize()
    return nc


def kernel(x: np.ndarray, kernels: np.ndarray) -> np.ndarray:
    assert x.shape == (B, T, C) and kernels.shape == (K, 1, C)
    x = np.ascontiguousarray(x, dtype=np.float32)
    ws1, ws2 = _build_ws(np.asarray(kernels, dtype=np.float32))
    nc = build_nc(ws1, ws2)
    in_maps = [{"x": x[i * B_LOCAL:(i + 1) * B_LOCAL]} for i in range(N_CORES)]
    res = run_bass_kernel_spmd(nc, in_maps, core_ids=list(range(N_CORES)))
    return np.concatenate([r["y"] for r in res.results], axis=0)
